# revision 41
# baseline (speedup 1.0000x reference)
"""Mixture-of-Softmaxes kernel for 8 Trainium2 NeuronCores.

Strategy: tensor-parallel over the vocab dimension (V=100000 -> 12500/core).
Each core computes all B rows for its vocab shard. Per 128-row block:
per-head logits via fp8 DoubleRow matmuls (K=256 in one pass), exp on
ScalarE (the pacing engine), per-head row-sums on DVE (cheap fused
tensor_scalar accumulate), ONE [128,4] AllReduce per block of the softmax
denominators, then a fused scalar_tensor_tensor mixture on DVE that
accumulates in-place into the head-3 ring slot.  Output gathered on host.

Key structure vs the naive version:
- emb is *streamed* from DRAM per block (chunk-major loop: each emb chunk
  is used by all 4 heads back to back), freeing SBUF for a 13-slot ring of
  exp tiles so the per-block collective + mixture fully overlap the next
  block's compute.
- 8 AllReduces total (one [128,4] per block) instead of 32; the post-AR
  scale w = pi/S is computed on GPSIMD so no DVE FIFO slot ever waits on
  the collective except the mixture itself.
- Mixture ops for block i are emitted interleaved into block i+1's chunk
  stream so DVE row-sums (which feed the next AR) are not head-of-line
  blocked behind AR-dependent ops.
- projT stays resident in SBUF (no DRAM spill); fp8 copy written directly
  by the tanh activation.

Host-side prep: inputs transposed (contraction dim -> partitions); emb
pre-scaled by 16 and cast to fp8e4 (descaled for free via the exp's scale
argument); x/proj_mat/mix_mat in bf16.
"""

import numpy as np
import ml_dtypes

import concourse.bass as bass
import concourse.mybir as mybir
import concourse.tile as tile
from concourse import bacc
from concourse.bass_utils import run_bass_kernel_spmd
from concourse.bass_interp import get_hw_module

B, H, D, V = 1024, 4, 256, 100000
N_CORES = 8
V_S = V // N_CORES          # 12500 vocab entries per core
KT = D // 128               # 2 contraction k-tiles
N_BBLK = B // 128           # 8 b-blocks
H0W, H1W = 6144, 6356       # uneven halves of V_S (3x2048 | 3x2048+212)
SLOTW = H1W                 # ring slot width
E_SLOTS = 13                # exp ring slots (halves)
# (offset, width): two [128,2048] psum buffers ping-pong (global parity)
CHUNKS0 = [(0, 2048), (2048, 2048), (4096, 2048)]
CHUNKS1 = [(0, 2048), (2048, 2048), (4096, 2048), (6144, 212)]
NCH = len(CHUNKS0) + len(CHUNKS1)  # chunks (= accum cols) per head

FP8 = True                  # fp8e4 DoubleRow matmul for the big GEMM
DVE_SUMS = False            # row-sums on DVE (else ScalarE accum_out)
EMB_SCALE = 16.0            # host pre-scale of emb (undone in exp's scale)

F32 = mybir.dt.float32
BF16 = mybir.dt.bfloat16
FP8E4 = mybir.dt.float8e4

_RUN_KWARGS = {}  # test harness may set trace/tmpdir here
_CACHE = {}


def _build():
    nc = bacc.Bacc("TRN2", target_bir_lowering=False, debug=False,
                   num_devices=N_CORES)
    xT = nc.dram_tensor("xT", [D, B], BF16, kind="ExternalInput").ap()
    pmT = nc.dram_tensor("pmT", [D, H * D], BF16, kind="ExternalInput").ap()
    mmT = nc.dram_tensor("mmT", [D, H], BF16, kind="ExternalInput").ap()
    if FP8:
        embT = nc.dram_tensor("embT", [128, KT * V_S], FP8E4,
                              kind="ExternalInput").ap()
    else:
        embT = nc.dram_tensor("embT", [D, V_S], BF16, kind="ExternalInput").ap()
    out = nc.dram_tensor("out", [B, V_S], BF16, kind="ExternalOutput").ap()

    with tile.TileContext(nc) as tc:
        _body(tc, xT, pmT, mmT, embT, out)
        tc._pool_ctx.close()

    nc.compile()
    nc.m = get_hw_module(nc.m)
    return nc


def _body(tc, xT, pmT, mmT, embT, out):
    nc = tc.nc
    Exp = mybir.ActivationFunctionType.Exp
    Tanh = mybir.ActivationFunctionType.Tanh
    add = mybir.AluOpType.add
    mult = mybir.AluOpType.mult
    divide = mybir.AluOpType.divide

    import contextlib
    ctx = contextlib.ExitStack()
    tc._pool_ctx = ctx
    singles = ctx.enter_context(tc.tile_pool(name="singles", bufs=1))
    work = ctx.enter_context(tc.tile_pool(name="work", bufs=2))
    ering = ctx.enter_context(tc.tile_pool(name="ering", bufs=E_SLOTS))
    estream = ctx.enter_context(tc.tile_pool(name="estream", bufs=2))
    psum = ctx.enter_context(tc.tile_pool(name="psum", bufs=2, space="PSUM"))
    dram = ctx.enter_context(tc.tile_pool(name="dram", bufs=2, space="DRAM"))

    # ---- warm up the CC stream: the first ~3 collectives otherwise pay
    # ~20-40us of one-time setup on the critical path ----
    zz = work.tile([128, H], F32, tag="zz", name="zz")
    nc.gpsimd.memset(zz, 0.0)
    for wi in range(3):
        warm_in = dram.tile([128, H], F32, tag=f"wrmin{wi}",
                            name=f"wrmin{wi}", bufs=1)
        warm_out = dram.tile([128, H], F32, tag=f"wrmout{wi}",
                             name=f"wrmout{wi}", bufs=1)
        nc.gpsimd.dma_start(out=warm_in[:], in_=zz)
        nc.gpsimd.collective_compute(
            "AllReduce", add,
            replica_groups=[list(range(N_CORES))],
            ins=[warm_in.opt()], outs=[warm_out.opt()],
        )

    # ---- prologue: resident inputs ----
    sb_xT, sb_pmT, sb_mmT = [], [], []
    for k in range(KT):
        t = singles.tile([128, B], BF16, tag=f"xT{k}", name=f"xT{k}")
        nc.sync.dma_start(out=t, in_=xT[k * 128:(k + 1) * 128, :])
        sb_xT.append(t)
        t = singles.tile([128, H * D], BF16, tag=f"pmT{k}", name=f"pmT{k}")
        nc.sync.dma_start(out=t, in_=pmT[k * 128:(k + 1) * 128, :])
        sb_pmT.append(t)
        t = singles.tile([128, H], BF16, tag=f"mmT{k}", name=f"mmT{k}")
        nc.sync.dma_start(out=t, in_=mmT[k * 128:(k + 1) * 128, :])
        sb_mmT.append(t)

    ps_parity = [0]

    def next_ps():
        pstag = "psA" if ps_parity[0] % 2 == 0 else "psB"
        ps_parity[0] += 1
        return psum.tile([128, 2048], F32, tag=pstag, name=pstag, bufs=1)

    # ---- projT = tanh(proj_mat @ x.T), resident (fp8 interleaved or bf16)
    if FP8:
        proj = [singles.tile([128, KT, B], FP8E4, tag=f"pj{h}", name=f"pj{h}")
                for h in range(H)]
    else:
        proj = [[singles.tile([128, B], BF16, tag=f"pj{h}_{kd}",
                              name=f"pj{h}_{kd}") for kd in range(KT)]
                for h in range(H)]
    for h in range(H):
        for kd in range(KT):
            for bs in range(B // 512):
                ps = next_ps()
                for kc in range(KT):
                    nc.tensor.matmul(
                        ps[:, :512],
                        sb_pmT[kc][:, h * D + kd * 128: h * D + (kd + 1) * 128],
                        sb_xT[kc][:, bs * 512:(bs + 1) * 512],
                        start=(kc == 0), stop=(kc == KT - 1),
                    )
                dst = (proj[h][:, kd, bs * 512:(bs + 1) * 512] if FP8
                       else proj[h][kd][:, bs * 512:(bs + 1) * 512])
                nc.scalar.activation(out=dst, in_=ps[:, :512], func=Tanh)

    # ---- pi[b, h] = softmax_h(x @ mix_mat.T) per b-block ----
    sb_pi = []
    for i in range(N_BBLK):
        ps = next_ps()
        for kc in range(KT):
            nc.tensor.matmul(
                ps[:, :H],
                sb_xT[kc][:, i * 128:(i + 1) * 128],
                sb_mmT[kc],
                start=(kc == 0), stop=(kc == KT - 1),
            )
        m = work.tile([128, 1], F32, tag="pim", name="pim")
        nc.vector.tensor_reduce(out=m, in_=ps[:, :H],
                                axis=mybir.AxisListType.X,
                                op=mybir.AluOpType.max)
        negm = work.tile([128, 1], F32, tag="pinegm", name="pinegm")
        nc.vector.tensor_scalar_mul(negm, m, -1.0)
        e = work.tile([128, H], F32, tag="pie", name="pie")
        nc.scalar.activation(out=e, in_=ps[:, :H], func=Exp, bias=negm)
        s = work.tile([128, 1], F32, tag="pis", name="pis")
        nc.vector.tensor_reduce(out=s, in_=e, axis=mybir.AxisListType.X,
                                op=add)
        rs = work.tile([128, 1], F32, tag="pirs", name="pirs")
        nc.vector.reciprocal(rs, s)
        pi = singles.tile([128, H], F32, tag=f"pi{i}", name=f"pi{i}")
        nc.vector.tensor_scalar_mul(pi, e, rs)
        sb_pi.append(pi)

    # ---- main loop over b-blocks (head-outer within each half) ----
    exp_scale = (1.0 / EMB_SCALE) if FP8 else 1.0

    def load_half(q):
        """DMA one emb half into SBUF (used by all 4 heads of the block)."""
        qoff = H0W if q else 0
        qw = H1W if q else H0W
        if FP8:
            eh = estream.tile([128, KT, SLOTW], FP8E4, tag="eh", name="eh")
            for kd in range(KT):
                nc.sync.dma_start(
                    out=eh[:, kd, :qw],
                    in_=embT[:, kd * V_S + qoff: kd * V_S + qoff + qw])
        else:
            eh = [estream.tile([128, SLOTW], BF16, tag=f"eh{kd}",
                               name=f"eh{kd}") for kd in range(KT)]
            for kd in range(KT):
                nc.sync.dma_start(
                    out=eh[kd][:, :qw],
                    in_=embT[kd * 128:(kd + 1) * 128, qoff:qoff + qw])
        return eh

    def emit_head_run(i, h, q, ci_base, chunks, eh, eq):
        """matmuls+exp(+sums) for one head over one emb half (weights
        loaded once)."""
        for cidx, (c0, cw) in enumerate(chunks):
            ci = ci_base + cidx
            ps = next_ps()
            for ns in range((cw + 511) // 512):
                n0 = c0 + ns * 512
                nw = min(512, c0 + cw - n0)
                if FP8:
                    nc.tensor.matmul(
                        ps[:, ns * 512:ns * 512 + nw],
                        proj[h][:, :, i * 128:(i + 1) * 128],
                        eh[:, :, n0:n0 + nw],
                        start=True, stop=True,
                        perf_mode=mybir.MatmulPerfMode.DoubleRow,
                    )
                else:
                    for kc in range(KT):
                        nc.tensor.matmul(
                            ps[:, ns * 512:ns * 512 + nw],
                            proj[h][kc][:, i * 128:(i + 1) * 128],
                            eh[kc][:, n0:n0 + nw],
                            start=(kc == 0), stop=(kc == KT - 1),
                        )
            nc.scalar.activation(out=eq[h][:, c0:c0 + cw], in_=ps[:, :cw],
                                 func=Exp, scale=exp_scale,
                                 accum_out=sums_t[i % 2][:, h * NCH + ci:
                                                         h * NCH + ci + 1])

    sums_t = [work.tile([128, NCH * H], F32, tag=f"sums{j}",
                        name=f"sums{j}") for j in range(2)]

    pending = []  # deferred DVE mixture ops of the previous block

    def drain(n):
        for _ in range(min(n, len(pending))):
            pending.pop(0)()

    eh0_next = None
    for i in range(N_BBLK - 1):
        # --- half 0 (prefetched during the previous block's q1) ---
        eh0 = eh0_next if eh0_next is not None else load_half(0)
        eq0 = [ering.tile([128, SLOTW], BF16, tag="e", name=f"e{h}q0")
               for h in range(H)]
        eh1 = load_half(1)  # issue q1's DMA now: lands well before use
        for h in range(H):
            emit_head_run(i, h, 0, 0, CHUNKS0, eh0, eq0)
            drain((3, 2, 2, 2)[h])
        # --- half 1 ---
        eq1 = [ering.tile([128, SLOTW], BF16, tag="e", name=f"e{h}q1")
               for h in range(H)]
        eh0_next = load_half(0)  # prefetch next block's q0
        for h in range(H):
            emit_head_run(i, h, 1, len(CHUNKS0), CHUNKS1, eh1, eq1)
            drain((2, 2, 2, 2)[h])
        drain(len(pending))  # safety: should be empty already

        # --- block-end: local denominators -> AllReduce -> w = pi/S ---
        s_loc = work.tile([128, H], F32, tag="sloc", name="sloc")
        for h in range(H):
            nc.vector.tensor_reduce(
                out=s_loc[:, h:h + 1],
                in_=sums_t[i % 2][:, h * NCH:(h + 1) * NCH],
                axis=mybir.AxisListType.X, op=add)
        cc_in = dram.tile([128, H], F32, tag="ccin", name="ccin")
        cc_out = dram.tile([128, H], F32, tag="ccout", name="ccout")
        nc.gpsimd.dma_start(out=cc_in[:], in_=s_loc)
        nc.gpsimd.collective_compute(
            "AllReduce", add,
            replica_groups=[list(range(N_CORES))],
            ins=[cc_in.opt()], outs=[cc_out.opt()],
        )
        s_glob = work.tile([128, H], F32, tag="sglob", name="sglob")
        nc.gpsimd.dma_start(out=s_glob, in_=cc_out[:])
        # w = pi / S_glob on DVE, deferred (first AR-dependent ops in the
        # DVE FIFO are emitted ~1 chunk into the next block)
        rS = work.tile([128, H], F32, tag="rS", name="rS")
        w = work.tile([128, H], F32, tag="w", name="w")

        def op_w(rS=rS, w=w, s_glob=s_glob, pi=sb_pi[i]):
            nc.vector.reciprocal(rS, s_glob)
            nc.vector.tensor_mul(w, pi, rS)

        # --- mixture for this block: deferred into next block's stream ---
        # All in-place on the ring slots: scale each e_h by w_h (4x mode),
        # then accumulate into the head-3 slot with tensor_tensor adds (2x).
        def make_mix(i, w, eq, qw, goff):
            acc = eq[H - 1]

            def mul(h):
                def op():
                    nc.vector.tensor_scalar_mul(eq[h][:, :qw], eq[h][:, :qw],
                                                w[:, h:h + 1])
                return op

            def madd(h):
                def op():
                    nc.vector.tensor_tensor(out=acc[:, :qw], in0=acc[:, :qw],
                                            in1=eq[h][:, :qw], op=add)
                return op

            def op_dma():
                nc.sync.dma_start(
                    out=out[i * 128:(i + 1) * 128, goff:goff + qw],
                    in_=acc[:, :qw])
            return [mul(0), mul(H - 1), madd(0), mul(1), madd(1),
                    mul(2), madd(2), op_dma]

        # out-DMAs (sync HWDGE) sit at drain positions where their
        # mixture-waits are already satisfied at queue-head: no
        # head-of-line blocking of the emb stream
        pending = ([op_w] + make_mix(i, w, eq0, H0W, 0)
                   + make_mix(i, w, eq1, H1W, H0W))

    # ---- final block: head-major with per-head AllReduce, so only the
    # last head's collective + one scale/add/DMA remain after the last exp
    i = N_BBLK - 1
    eh0 = eh0_next
    eh1 = load_half(1)
    eq0, eq1 = [None] * H, [None] * H
    mix7 = []  # deferred per-head mixture ops (keep reduces at FIFO head)

    def emit_mix7(h, w7):
        for eq, qw in ((eq0, H0W), (eq1, H1W)):
            def op_mul(eq=eq, qw=qw, h=h, w7=w7):
                nc.vector.tensor_scalar_mul(eq[h][:, :qw], eq[h][:, :qw],
                                            w7)
            mix7.append(op_mul)
        if h > 0:
            for eq, qw in ((eq0, H0W), (eq1, H1W)):
                def op_add(eq=eq, qw=qw, h=h):
                    nc.vector.tensor_tensor(out=eq[h][:, :qw],
                                            in0=eq[h][:, :qw],
                                            in1=eq[h - 1][:, :qw], op=add)
                mix7.append(op_add)

    for h in range(H):
        eq0[h] = ering.tile([128, SLOTW], BF16, tag="e", name=f"f{h}q0")
        eq1[h] = ering.tile([128, SLOTW], BF16, tag="e", name=f"f{h}q1")
        emit_head_run(i, h, 0, 0, CHUNKS0, eh0, eq0)
        emit_head_run(i, h, 1, len(CHUNKS0), CHUNKS1, eh1, eq1)
        drain(5)  # block 6's deferred mixture
        # issue this head's AllReduce trigger chain immediately; the
        # mixture ops run behind later heads' reduces in the DVE FIFO
        sl = work.tile([128, 1], F32, tag=f"sl7_{h}", name=f"sl7_{h}",
                       bufs=1)
        nc.vector.tensor_reduce(
            out=sl, in_=sums_t[i % 2][:, h * NCH:(h + 1) * NCH],
            axis=mybir.AxisListType.X, op=add)
        cc7i = dram.tile([128, 1], F32, tag=f"cc7i{h}", name=f"cc7i{h}",
                         bufs=1)
        cc7o = dram.tile([128, 1], F32, tag=f"cc7o{h}", name=f"cc7o{h}",
                         bufs=1)
        nc.gpsimd.dma_start(out=cc7i[:], in_=sl)
        nc.gpsimd.collective_compute(
            "AllReduce", add,
            replica_groups=[list(range(N_CORES))],
            ins=[cc7i.opt()], outs=[cc7o.opt()],
        )
        sg = work.tile([128, 1], F32, tag=f"sg7{h}", name=f"sg7{h}", bufs=1)
        nc.gpsimd.dma_start(out=sg, in_=cc7o[:])
        rs7 = work.tile([128, 1], F32, tag=f"rs7{h}", name=f"rs7{h}", bufs=1)
        w7 = work.tile([128, 1], F32, tag=f"w7{h}", name=f"w7{h}", bufs=1)

        def op_w7(rs7=rs7, w7=w7, sg=sg, h=h):
            nc.vector.reciprocal(rs7, sg)
            nc.vector.tensor_mul(w7, sb_pi[N_BBLK - 1][:, h:h + 1], rs7)
        mix7.append(op_w7)
        emit_mix7(h, w7)
        # flush older heads' mixture ops (their ARs are already done);
        # keep only this head's ops pending so the next head's reduce
        # stays near the DVE FIFO head
        if h < H - 1:
            keep = 3 if h == 0 else 5
            while len(mix7) > keep:
                mix7.pop(0)()
    for op in mix7:
        op()
    nc.sync.dma_start(out=out[i * 128:(i + 1) * 128, 0:H0W],
                      in_=eq0[H - 1][:, :H0W])
    nc.sync.dma_start(out=out[i * 128:(i + 1) * 128, H0W:V_S],
                      in_=eq1[H - 1][:, :H1W])
    drain(len(pending))  # safety


def _get_nc():
    if "nc" not in _CACHE:
        _CACHE["nc"] = _build()
    return _CACHE["nc"]


def kernel(x, proj_mat, mix_mat, emb):
    nc = _get_nc()
    bf = ml_dtypes.bfloat16
    xT = np.ascontiguousarray(x.astype(bf).T)
    pmT = np.ascontiguousarray(proj_mat.astype(bf).T)
    mmT = np.ascontiguousarray(mix_mat.astype(bf).T)
    in_maps = []
    for c in range(N_CORES):
        shard = emb[c * V_S:(c + 1) * V_S]
        if FP8:
            # [dl, kd*V_S + v] = emb[v, kd*128+dl] * EMB_SCALE, fp8e4
            e16 = (shard.T * EMB_SCALE).astype(ml_dtypes.float8_e4m3)
            embT = np.ascontiguousarray(
                e16.reshape(KT, 128, V_S).transpose(1, 0, 2).reshape(
                    128, KT * V_S))
        else:
            embT = np.ascontiguousarray(shard.astype(bf).T)
        in_maps.append({"xT": xT, "pmT": pmT, "mmT": mmT, "embT": embT})
    res = run_bass_kernel_spmd(nc, in_maps, list(range(N_CORES)),
                               **_RUN_KWARGS)
    _CACHE["last_result"] = res
    return np.concatenate(
        [res.results[c]["out"].astype(np.float32) for c in range(N_CORES)],
        axis=1)


# revision 43
# speedup vs baseline: 1.0211x; 1.0211x over previous
"""Mixture-of-Softmaxes kernel for 8 Trainium2 NeuronCores.

Strategy: tensor-parallel over the vocab dimension (V=100000 -> 12500/core).
Each core computes all B rows for its vocab shard. Per 128-row block:
per-head logits via fp8 DoubleRow matmuls (K=256 in one pass), exp on
ScalarE (the pacing engine), per-head row-sums on DVE (cheap fused
tensor_scalar accumulate), ONE [128,4] AllReduce per block of the softmax
denominators, then a fused scalar_tensor_tensor mixture on DVE that
accumulates in-place into the head-3 ring slot.  Output gathered on host.

Key structure vs the naive version:
- emb is *streamed* from DRAM per block (chunk-major loop: each emb chunk
  is used by all 4 heads back to back), freeing SBUF for a 13-slot ring of
  exp tiles so the per-block collective + mixture fully overlap the next
  block's compute.
- 8 AllReduces total (one [128,4] per block) instead of 32; the post-AR
  scale w = pi/S is computed on GPSIMD so no DVE FIFO slot ever waits on
  the collective except the mixture itself.
- Mixture ops for block i are emitted interleaved into block i+1's chunk
  stream so DVE row-sums (which feed the next AR) are not head-of-line
  blocked behind AR-dependent ops.
- projT stays resident in SBUF (no DRAM spill); fp8 copy written directly
  by the tanh activation.

Host-side prep: inputs transposed (contraction dim -> partitions); emb
pre-scaled by 16 and cast to fp8e4 (descaled for free via the exp's scale
argument); x/proj_mat/mix_mat in bf16.
"""

import numpy as np
import ml_dtypes

import concourse.bass as bass
import concourse.mybir as mybir
import concourse.tile as tile
from concourse import bacc
from concourse.bass_utils import run_bass_kernel_spmd
from concourse.bass_interp import get_hw_module

B, H, D, V = 1024, 4, 256, 100000
N_CORES = 8
V_S = V // N_CORES          # 12500 vocab entries per core
KT = D // 128               # 2 contraction k-tiles
N_BBLK = B // 128           # 8 b-blocks
H0W, H1W = 6144, 6356       # uneven halves of V_S (3x2048 | 3x2048+212)
SLOTW = H1W                 # ring slot width
E_SLOTS = 13                # exp ring slots (halves)
# (offset, width): two [128,2048] psum buffers ping-pong (global parity)
CHUNKS0 = [(0, 2048), (2048, 2048), (4096, 2048)]
CHUNKS1 = [(0, 2048), (2048, 2048), (4096, 2048), (6144, 212)]
NCH = len(CHUNKS0) + len(CHUNKS1)  # chunks (= accum cols) per head

FP8 = True                  # fp8e4 DoubleRow matmul for the big GEMM
DVE_SUMS = False            # row-sums on DVE (else ScalarE accum_out)
EMB_SCALE = 16.0            # host pre-scale of emb (undone in exp's scale)

F32 = mybir.dt.float32
BF16 = mybir.dt.bfloat16
FP8E4 = mybir.dt.float8e4

_RUN_KWARGS = {}  # test harness may set trace/tmpdir here
_CACHE = {}


def _build():
    nc = bacc.Bacc("TRN2", target_bir_lowering=False, debug=False,
                   num_devices=N_CORES)
    xT = nc.dram_tensor("xT", [D, B], BF16, kind="ExternalInput").ap()
    pmT = nc.dram_tensor("pmT", [D, H * D], BF16, kind="ExternalInput").ap()
    mmT = nc.dram_tensor("mmT", [D, H], BF16, kind="ExternalInput").ap()
    if FP8:
        embT = nc.dram_tensor("embT", [128, KT * V_S], FP8E4,
                              kind="ExternalInput").ap()
    else:
        embT = nc.dram_tensor("embT", [D, V_S], BF16, kind="ExternalInput").ap()
    out = nc.dram_tensor("out", [B, V_S], BF16, kind="ExternalOutput").ap()

    with tile.TileContext(nc) as tc:
        _body(tc, xT, pmT, mmT, embT, out)
        tc._pool_ctx.close()

    nc.compile()
    nc.m = get_hw_module(nc.m)
    return nc


def _body(tc, xT, pmT, mmT, embT, out):
    nc = tc.nc
    Exp = mybir.ActivationFunctionType.Exp
    Tanh = mybir.ActivationFunctionType.Tanh
    add = mybir.AluOpType.add
    mult = mybir.AluOpType.mult
    divide = mybir.AluOpType.divide

    import contextlib
    ctx = contextlib.ExitStack()
    tc._pool_ctx = ctx
    singles = ctx.enter_context(tc.tile_pool(name="singles", bufs=1))
    work = ctx.enter_context(tc.tile_pool(name="work", bufs=2))
    ering = ctx.enter_context(tc.tile_pool(name="ering", bufs=E_SLOTS))
    estream = ctx.enter_context(tc.tile_pool(name="estream", bufs=2))
    psum = ctx.enter_context(tc.tile_pool(name="psum", bufs=2, space="PSUM"))
    dram = ctx.enter_context(tc.tile_pool(name="dram", bufs=2, space="DRAM"))

    # ---- warm up the CC stream: the first ~3 collectives otherwise pay
    # ~20-40us of one-time setup on the critical path ----
    zz = work.tile([128, H], F32, tag="zz", name="zz")
    nc.gpsimd.memset(zz, 0.0)
    for wi in range(3):
        warm_in = dram.tile([128, H], F32, tag=f"wrmin{wi}",
                            name=f"wrmin{wi}", bufs=1)
        warm_out = dram.tile([128, H], F32, tag=f"wrmout{wi}",
                             name=f"wrmout{wi}", bufs=1)
        nc.gpsimd.dma_start(out=warm_in[:], in_=zz)
        nc.gpsimd.collective_compute(
            "AllReduce", add,
            replica_groups=[list(range(N_CORES))],
            ins=[warm_in.opt()], outs=[warm_out.opt()],
        )

    # ---- prologue: resident inputs ----
    sb_xT, sb_pmT, sb_mmT = [], [], []
    for k in range(KT):
        t = singles.tile([128, B], BF16, tag=f"xT{k}", name=f"xT{k}")
        nc.sync.dma_start(out=t, in_=xT[k * 128:(k + 1) * 128, :])
        sb_xT.append(t)
        t = singles.tile([128, H * D], BF16, tag=f"pmT{k}", name=f"pmT{k}")
        nc.sync.dma_start(out=t, in_=pmT[k * 128:(k + 1) * 128, :])
        sb_pmT.append(t)
        t = singles.tile([128, H], BF16, tag=f"mmT{k}", name=f"mmT{k}")
        nc.sync.dma_start(out=t, in_=mmT[k * 128:(k + 1) * 128, :])
        sb_mmT.append(t)

    ps_parity = [0]

    def next_ps():
        pstag = "psA" if ps_parity[0] % 2 == 0 else "psB"
        ps_parity[0] += 1
        return psum.tile([128, 2048], F32, tag=pstag, name=pstag, bufs=1)

    # ---- projT = tanh(proj_mat @ x.T), resident (fp8 interleaved or bf16)
    if FP8:
        proj = [singles.tile([128, KT, B], FP8E4, tag=f"pj{h}", name=f"pj{h}")
                for h in range(H)]
    else:
        proj = [[singles.tile([128, B], BF16, tag=f"pj{h}_{kd}",
                              name=f"pj{h}_{kd}") for kd in range(KT)]
                for h in range(H)]
    for h in range(H):
        for kd in range(KT):
            for bs in range(B // 512):
                ps = next_ps()
                for kc in range(KT):
                    nc.tensor.matmul(
                        ps[:, :512],
                        sb_pmT[kc][:, h * D + kd * 128: h * D + (kd + 1) * 128],
                        sb_xT[kc][:, bs * 512:(bs + 1) * 512],
                        start=(kc == 0), stop=(kc == KT - 1),
                    )
                dst = (proj[h][:, kd, bs * 512:(bs + 1) * 512] if FP8
                       else proj[h][kd][:, bs * 512:(bs + 1) * 512])
                nc.scalar.activation(out=dst, in_=ps[:, :512], func=Tanh)

    # ---- pi[b, h] = softmax_h(x @ mix_mat.T) per b-block ----
    sb_pi = []
    for i in range(N_BBLK):
        ps = next_ps()
        for kc in range(KT):
            nc.tensor.matmul(
                ps[:, :H],
                sb_xT[kc][:, i * 128:(i + 1) * 128],
                sb_mmT[kc],
                start=(kc == 0), stop=(kc == KT - 1),
            )
        m = work.tile([128, 1], F32, tag="pim", name="pim")
        nc.vector.tensor_reduce(out=m, in_=ps[:, :H],
                                axis=mybir.AxisListType.X,
                                op=mybir.AluOpType.max)
        negm = work.tile([128, 1], F32, tag="pinegm", name="pinegm")
        nc.vector.tensor_scalar_mul(negm, m, -1.0)
        e = work.tile([128, H], F32, tag="pie", name="pie")
        nc.scalar.activation(out=e, in_=ps[:, :H], func=Exp, bias=negm)
        s = work.tile([128, 1], F32, tag="pis", name="pis")
        nc.vector.tensor_reduce(out=s, in_=e, axis=mybir.AxisListType.X,
                                op=add)
        rs = work.tile([128, 1], F32, tag="pirs", name="pirs")
        nc.vector.reciprocal(rs, s)
        pi = singles.tile([128, H], F32, tag=f"pi{i}", name=f"pi{i}")
        nc.vector.tensor_scalar_mul(pi, e, rs)
        sb_pi.append(pi)

    # ---- main loop over b-blocks (head-outer within each half) ----
    exp_scale = (1.0 / EMB_SCALE) if FP8 else 1.0

    def load_half(q):
        """DMA one emb half into SBUF (used by all 4 heads of the block)."""
        qoff = H0W if q else 0
        qw = H1W if q else H0W
        if FP8:
            eh = estream.tile([128, KT, SLOTW], FP8E4, tag="eh", name="eh")
            for kd in range(KT):
                nc.sync.dma_start(
                    out=eh[:, kd, :qw],
                    in_=embT[:, kd * V_S + qoff: kd * V_S + qoff + qw])
        else:
            eh = [estream.tile([128, SLOTW], BF16, tag=f"eh{kd}",
                               name=f"eh{kd}") for kd in range(KT)]
            for kd in range(KT):
                nc.sync.dma_start(
                    out=eh[kd][:, :qw],
                    in_=embT[kd * 128:(kd + 1) * 128, qoff:qoff + qw])
        return eh

    def emit_head_run(i, h, q, ci_base, chunks, eh, eq):
        """matmuls+exp(+sums) for one head over one emb half (weights
        loaded once)."""
        for cidx, (c0, cw) in enumerate(chunks):
            ci = ci_base + cidx
            ps = next_ps()
            for ns in range((cw + 511) // 512):
                n0 = c0 + ns * 512
                nw = min(512, c0 + cw - n0)
                if FP8:
                    nc.tensor.matmul(
                        ps[:, ns * 512:ns * 512 + nw],
                        proj[h][:, :, i * 128:(i + 1) * 128],
                        eh[:, :, n0:n0 + nw],
                        start=True, stop=True,
                        perf_mode=mybir.MatmulPerfMode.DoubleRow,
                    )
                else:
                    for kc in range(KT):
                        nc.tensor.matmul(
                            ps[:, ns * 512:ns * 512 + nw],
                            proj[h][kc][:, i * 128:(i + 1) * 128],
                            eh[kc][:, n0:n0 + nw],
                            start=(kc == 0), stop=(kc == KT - 1),
                        )
            nc.scalar.activation(out=eq[h][:, c0:c0 + cw], in_=ps[:, :cw],
                                 func=Exp, scale=exp_scale,
                                 accum_out=sums_t[i % 2][:, h * NCH + ci:
                                                         h * NCH + ci + 1])

    sums_t = [work.tile([128, NCH * H], F32, tag=f"sums{j}",
                        name=f"sums{j}") for j in range(2)]

    pending = []  # deferred DVE mixture ops of the previous block

    def drain(n):
        for _ in range(min(n, len(pending))):
            pending.pop(0)()

    eh0_next = None
    for i in range(N_BBLK - 1):
        # --- half 0 (prefetched during the previous block's q1) ---
        eh0 = eh0_next if eh0_next is not None else load_half(0)
        eq0 = [ering.tile([128, SLOTW], BF16, tag="e", name=f"e{h}q0")
               for h in range(H)]
        eh1 = load_half(1)  # issue q1's DMA now: lands well before use
        for h in range(H):
            emit_head_run(i, h, 0, 0, CHUNKS0, eh0, eq0)
            drain((3, 2, 2, 2)[h])
        # --- half 1: reduces are emitted before the drains so the next
        # AllReduce trigger never queues behind AR-dependent mixture ops
        eq1 = [ering.tile([128, SLOTW], BF16, tag="e", name=f"e{h}q1")
               for h in range(H)]
        eh0_next = load_half(0)  # prefetch next block's q0
        s_loc = work.tile([128, H], F32, tag="sloc", name="sloc")
        cc_in = dram.tile([128, H], F32, tag="ccin", name="ccin")
        cc_out = dram.tile([128, H], F32, tag="ccout", name="ccout")
        for h in range(H):
            emit_head_run(i, h, 1, len(CHUNKS0), CHUNKS1, eh1, eq1)
            nc.vector.tensor_reduce(
                out=s_loc[:, h:h + 1],
                in_=sums_t[i % 2][:, h * NCH:(h + 1) * NCH],
                axis=mybir.AxisListType.X, op=add)
            if h == H - 1:
                nc.gpsimd.dma_start(out=cc_in[:], in_=s_loc)
                nc.gpsimd.collective_compute(
                    "AllReduce", add,
                    replica_groups=[list(range(N_CORES))],
                    ins=[cc_in.opt()], outs=[cc_out.opt()],
                )
            drain((2, 2, 2, 2)[h])
        drain(len(pending))  # safety: should be empty already
        s_glob = work.tile([128, H], F32, tag="sglob", name="sglob")
        nc.gpsimd.dma_start(out=s_glob, in_=cc_out[:])
        # w = pi / S_glob on DVE, deferred (first AR-dependent ops in the
        # DVE FIFO are emitted ~1 chunk into the next block)
        rS = work.tile([128, H], F32, tag="rS", name="rS")
        w = work.tile([128, H], F32, tag="w", name="w")

        def op_w(rS=rS, w=w, s_glob=s_glob, pi=sb_pi[i]):
            nc.vector.reciprocal(rS, s_glob)
            nc.vector.tensor_mul(w, pi, rS)

        # --- mixture for this block: deferred into next block's stream ---
        # All in-place on the ring slots: scale each e_h by w_h (4x mode),
        # then accumulate into the head-3 slot with tensor_tensor adds (2x).
        def make_mix(i, w, eq, qw, goff):
            acc = eq[H - 1]

            def mul(h):
                def op():
                    nc.vector.tensor_scalar_mul(eq[h][:, :qw], eq[h][:, :qw],
                                                w[:, h:h + 1])
                return op

            def madd(h):
                def op():
                    nc.vector.tensor_tensor(out=acc[:, :qw], in0=acc[:, :qw],
                                            in1=eq[h][:, :qw], op=add)
                return op

            def op_dma():
                nc.sync.dma_start(
                    out=out[i * 128:(i + 1) * 128, goff:goff + qw],
                    in_=acc[:, :qw])
            return [mul(0), mul(H - 1), madd(0), mul(1), madd(1),
                    mul(2), madd(2), op_dma]

        # out-DMAs (sync HWDGE) sit at drain positions where their
        # mixture-waits are already satisfied at queue-head: no
        # head-of-line blocking of the emb stream
        pending = ([op_w] + make_mix(i, w, eq0, H0W, 0)
                   + make_mix(i, w, eq1, H1W, H0W))

    # ---- final block: head-major with per-head AllReduce, so only the
    # last head's collective + one scale/add/DMA remain after the last exp
    i = N_BBLK - 1
    eh0 = eh0_next
    eh1 = load_half(1)
    eq0, eq1 = [None] * H, [None] * H
    mix7 = []  # deferred per-head mixture ops (keep reduces at FIFO head)

    def emit_mix7(h, w7):
        for eq, qw in ((eq0, H0W), (eq1, H1W)):
            def op_mul(eq=eq, qw=qw, h=h, w7=w7):
                nc.vector.tensor_scalar_mul(eq[h][:, :qw], eq[h][:, :qw],
                                            w7)
            mix7.append(op_mul)
        if h > 0:
            for eq, qw in ((eq0, H0W), (eq1, H1W)):
                def op_add(eq=eq, qw=qw, h=h):
                    nc.vector.tensor_tensor(out=eq[h][:, :qw],
                                            in0=eq[h][:, :qw],
                                            in1=eq[h - 1][:, :qw], op=add)
                mix7.append(op_add)

    for h in range(H):
        eq0[h] = ering.tile([128, SLOTW], BF16, tag="e", name=f"f{h}q0")
        eq1[h] = ering.tile([128, SLOTW], BF16, tag="e", name=f"f{h}q1")
        emit_head_run(i, h, 0, 0, CHUNKS0, eh0, eq0)
        emit_head_run(i, h, 1, len(CHUNKS0), CHUNKS1, eh1, eq1)
        # issue this head's AllReduce trigger chain immediately; the
        # mixture ops run behind later heads' reduces in the DVE FIFO
        sl = work.tile([128, 1], F32, tag=f"sl7_{h}", name=f"sl7_{h}",
                       bufs=1)
        nc.vector.tensor_reduce(
            out=sl, in_=sums_t[i % 2][:, h * NCH:(h + 1) * NCH],
            axis=mybir.AxisListType.X, op=add)
        cc7i = dram.tile([128, 1], F32, tag=f"cc7i{h}", name=f"cc7i{h}",
                         bufs=1)
        cc7o = dram.tile([128, 1], F32, tag=f"cc7o{h}", name=f"cc7o{h}",
                         bufs=1)
        nc.gpsimd.dma_start(out=cc7i[:], in_=sl)
        nc.gpsimd.collective_compute(
            "AllReduce", add,
            replica_groups=[list(range(N_CORES))],
            ins=[cc7i.opt()], outs=[cc7o.opt()],
        )
        sg = work.tile([128, 1], F32, tag=f"sg7{h}", name=f"sg7{h}", bufs=1)
        rs7 = work.tile([128, 1], F32, tag=f"rs7{h}", name=f"rs7{h}", bufs=1)
        w7 = work.tile([128, 1], F32, tag=f"w7{h}", name=f"w7{h}", bufs=1)

        def op_w7(rs7=rs7, w7=w7, sg=sg, cc7o=cc7o, h=h):
            nc.gpsimd.dma_start(out=sg, in_=cc7o[:])
            nc.vector.reciprocal(rs7, sg)
            nc.vector.tensor_mul(w7, sb_pi[N_BBLK - 1][:, h:h + 1], rs7)
        mix7.append(op_w7)
        emit_mix7(h, w7)
        drain(5)  # block 6's deferred mixture
        # flush older heads' mixture ops (their ARs are already done);
        # keep only this head's ops pending so the next head's reduce
        # stays near the DVE FIFO head
        if h < H - 1:
            keep = 3 if h == 0 else 5
            while len(mix7) > keep:
                mix7.pop(0)()
    for op in mix7:
        op()
    nc.sync.dma_start(out=out[i * 128:(i + 1) * 128, 0:H0W],
                      in_=eq0[H - 1][:, :H0W])
    nc.sync.dma_start(out=out[i * 128:(i + 1) * 128, H0W:V_S],
                      in_=eq1[H - 1][:, :H1W])
    drain(len(pending))  # safety


def _get_nc():
    if "nc" not in _CACHE:
        _CACHE["nc"] = _build()
    return _CACHE["nc"]


def kernel(x, proj_mat, mix_mat, emb):
    nc = _get_nc()
    bf = ml_dtypes.bfloat16
    xT = np.ascontiguousarray(x.astype(bf).T)
    pmT = np.ascontiguousarray(proj_mat.astype(bf).T)
    mmT = np.ascontiguousarray(mix_mat.astype(bf).T)
    in_maps = []
    for c in range(N_CORES):
        shard = emb[c * V_S:(c + 1) * V_S]
        if FP8:
            # [dl, kd*V_S + v] = emb[v, kd*128+dl] * EMB_SCALE, fp8e4
            e16 = (shard.T * EMB_SCALE).astype(ml_dtypes.float8_e4m3)
            embT = np.ascontiguousarray(
                e16.reshape(KT, 128, V_S).transpose(1, 0, 2).reshape(
                    128, KT * V_S))
        else:
            embT = np.ascontiguousarray(shard.astype(bf).T)
        in_maps.append({"xT": xT, "pmT": pmT, "mmT": mmT, "embT": embT})
    res = run_bass_kernel_spmd(nc, in_maps, list(range(N_CORES)),
                               **_RUN_KWARGS)
    _CACHE["last_result"] = res
    return np.concatenate(
        [res.results[c]["out"].astype(np.float32) for c in range(N_CORES)],
        axis=1)


# revision 44
# speedup vs baseline: 1.0245x; 1.0033x over previous
"""Mixture-of-Softmaxes kernel for 8 Trainium2 NeuronCores.

Strategy: tensor-parallel over the vocab dimension (V=100000 -> 12500/core).
Each core computes all B rows for its vocab shard. Per 128-row block:
per-head logits via fp8e4 DoubleRow matmuls (K=256 in one PE pass), exp on
ScalarE (the pacing engine, ~154G elem/s) with fused per-chunk row-sum
accumulators, ONE [128,4] AllReduce per block of the softmax denominators,
then an in-place DVE mixture (tensor_scalar_mul at 4x + tensor_tensor adds
at 2x) accumulating into the head-3 ring slot. Output gathered on host.

Key structure:
- emb is streamed from DRAM per half-block (head-outer loop: weights stay
  loaded across each head's chunks), freeing SBUF for a 13-slot ring of
  exp tiles so the per-block collective + mixture overlap the next block's
  compute; emb halves are prefetched one phase ahead.
- 8 AllReduces (one [128,4]/block) + 3 prologue warmup collectives (the
  first ~3 collectives pay one-time setup). The final block instead uses
  per-head [128,1] AllReduces so only the last head's collective + one
  scale/add/DMA remain after the last exp.
- DVE-FIFO discipline: row-sum reduces (which feed the next AllReduce
  trigger) are emitted ahead of the previous block's AR-gated mixture ops;
  big out-DMAs ride the sync HWDGE queue at positions where their waits
  are already satisfied, keeping both the emb stream and the gpsimd
  collective-staging queue free of head-of-line blocking.
- projT (tanh) is written by ScalarE directly as DoubleRow-interleaved
  fp8 [128, 2, B] tiles and stays SBUF-resident.

Host-side prep: inputs transposed (contraction dim -> partitions); emb
pre-scaled by 16 and cast to fp8e4 (descaled for free via the exp's scale
argument); x/proj_mat/mix_mat in bf16; output bf16 -> f32 cast + vocab
concat on host.
"""

import numpy as np
import ml_dtypes

import concourse.bass as bass
import concourse.mybir as mybir
import concourse.tile as tile
from concourse import bacc
from concourse.bass_utils import run_bass_kernel_spmd
from concourse.bass_interp import get_hw_module

B, H, D, V = 1024, 4, 256, 100000
N_CORES = 8
V_S = V // N_CORES          # 12500 vocab entries per core
KT = D // 128               # 2 contraction k-tiles
N_BBLK = B // 128           # 8 b-blocks
H0W, H1W = 6144, 6356       # uneven halves of V_S (3x2048 | 3x2048+212)
SLOTW = H1W                 # ring slot width
E_SLOTS = 13                # exp ring slots (halves)
# (offset, width): two [128,2048] psum buffers ping-pong (global parity)
CHUNKS0 = [(0, 2048), (2048, 2048), (4096, 2048)]
CHUNKS1 = [(0, 2048), (2048, 2048), (4096, 2048), (6144, 212)]
NCH = len(CHUNKS0) + len(CHUNKS1)  # chunks (= accum cols) per head

FP8 = True                  # fp8e4 DoubleRow matmul for the big GEMM
DVE_SUMS = False            # row-sums on DVE (else ScalarE accum_out)
EMB_SCALE = 16.0            # host pre-scale of emb (undone in exp's scale)

F32 = mybir.dt.float32
BF16 = mybir.dt.bfloat16
FP8E4 = mybir.dt.float8e4

_RUN_KWARGS = {}  # test harness may set trace/tmpdir here
_CACHE = {}


def _build():
    nc = bacc.Bacc("TRN2", target_bir_lowering=False, debug=False,
                   num_devices=N_CORES)
    xT = nc.dram_tensor("xT", [D, B], BF16, kind="ExternalInput").ap()
    pmT = nc.dram_tensor("pmT", [D, H * D], BF16, kind="ExternalInput").ap()
    mmT = nc.dram_tensor("mmT", [D, H], BF16, kind="ExternalInput").ap()
    if FP8:
        embT = nc.dram_tensor("embT", [128, KT * V_S], FP8E4,
                              kind="ExternalInput").ap()
    else:
        embT = nc.dram_tensor("embT", [D, V_S], BF16, kind="ExternalInput").ap()
    out = nc.dram_tensor("out", [B, V_S], BF16, kind="ExternalOutput").ap()

    with tile.TileContext(nc) as tc:
        _body(tc, xT, pmT, mmT, embT, out)
        tc._pool_ctx.close()

    nc.compile()
    nc.m = get_hw_module(nc.m)
    return nc


def _body(tc, xT, pmT, mmT, embT, out):
    nc = tc.nc
    Exp = mybir.ActivationFunctionType.Exp
    Tanh = mybir.ActivationFunctionType.Tanh
    add = mybir.AluOpType.add
    mult = mybir.AluOpType.mult
    divide = mybir.AluOpType.divide

    import contextlib
    ctx = contextlib.ExitStack()
    tc._pool_ctx = ctx
    singles = ctx.enter_context(tc.tile_pool(name="singles", bufs=1))
    work = ctx.enter_context(tc.tile_pool(name="work", bufs=2))
    ering = ctx.enter_context(tc.tile_pool(name="ering", bufs=E_SLOTS))
    estream = ctx.enter_context(tc.tile_pool(name="estream", bufs=2))
    psum = ctx.enter_context(tc.tile_pool(name="psum", bufs=2, space="PSUM"))
    dram = ctx.enter_context(tc.tile_pool(name="dram", bufs=2, space="DRAM"))

    # ---- warm up the CC stream: the first ~3 collectives otherwise pay
    # ~20-40us of one-time setup on the critical path ----
    zz = work.tile([128, H], F32, tag="zz", name="zz")
    nc.gpsimd.memset(zz, 0.0)
    for wi in range(3):
        warm_in = dram.tile([128, H], F32, tag=f"wrmin{wi}",
                            name=f"wrmin{wi}", bufs=1)
        warm_out = dram.tile([128, H], F32, tag=f"wrmout{wi}",
                             name=f"wrmout{wi}", bufs=1)
        nc.gpsimd.dma_start(out=warm_in[:], in_=zz)
        nc.gpsimd.collective_compute(
            "AllReduce", add,
            replica_groups=[list(range(N_CORES))],
            ins=[warm_in.opt()], outs=[warm_out.opt()],
        )

    # ---- prologue: resident inputs ----
    sb_xT, sb_pmT, sb_mmT = [], [], []
    for k in range(KT):
        t = singles.tile([128, B], BF16, tag=f"xT{k}", name=f"xT{k}")
        nc.sync.dma_start(out=t, in_=xT[k * 128:(k + 1) * 128, :])
        sb_xT.append(t)
        t = singles.tile([128, H * D], BF16, tag=f"pmT{k}", name=f"pmT{k}")
        nc.sync.dma_start(out=t, in_=pmT[k * 128:(k + 1) * 128, :])
        sb_pmT.append(t)
        t = singles.tile([128, H], BF16, tag=f"mmT{k}", name=f"mmT{k}")
        nc.sync.dma_start(out=t, in_=mmT[k * 128:(k + 1) * 128, :])
        sb_mmT.append(t)

    ps_parity = [0]

    def next_ps():
        pstag = "psA" if ps_parity[0] % 2 == 0 else "psB"
        ps_parity[0] += 1
        return psum.tile([128, 2048], F32, tag=pstag, name=pstag, bufs=1)

    # ---- projT = tanh(proj_mat @ x.T), resident (fp8 interleaved or bf16)
    if FP8:
        proj = [singles.tile([128, KT, B], FP8E4, tag=f"pj{h}", name=f"pj{h}")
                for h in range(H)]
    else:
        proj = [[singles.tile([128, B], BF16, tag=f"pj{h}_{kd}",
                              name=f"pj{h}_{kd}") for kd in range(KT)]
                for h in range(H)]
    for h in range(H):
        for kd in range(KT):
            for bs in range(B // 512):
                ps = next_ps()
                for kc in range(KT):
                    nc.tensor.matmul(
                        ps[:, :512],
                        sb_pmT[kc][:, h * D + kd * 128: h * D + (kd + 1) * 128],
                        sb_xT[kc][:, bs * 512:(bs + 1) * 512],
                        start=(kc == 0), stop=(kc == KT - 1),
                    )
                dst = (proj[h][:, kd, bs * 512:(bs + 1) * 512] if FP8
                       else proj[h][kd][:, bs * 512:(bs + 1) * 512])
                nc.scalar.activation(out=dst, in_=ps[:, :512], func=Tanh)

    # ---- pi[b, h] = softmax_h(x @ mix_mat.T) per b-block ----
    sb_pi = []
    for i in range(N_BBLK):
        ps = next_ps()
        for kc in range(KT):
            nc.tensor.matmul(
                ps[:, :H],
                sb_xT[kc][:, i * 128:(i + 1) * 128],
                sb_mmT[kc],
                start=(kc == 0), stop=(kc == KT - 1),
            )
        m = work.tile([128, 1], F32, tag="pim", name="pim")
        nc.vector.tensor_reduce(out=m, in_=ps[:, :H],
                                axis=mybir.AxisListType.X,
                                op=mybir.AluOpType.max)
        negm = work.tile([128, 1], F32, tag="pinegm", name="pinegm")
        nc.vector.tensor_scalar_mul(negm, m, -1.0)
        e = work.tile([128, H], F32, tag="pie", name="pie")
        nc.scalar.activation(out=e, in_=ps[:, :H], func=Exp, bias=negm)
        s = work.tile([128, 1], F32, tag="pis", name="pis")
        nc.vector.tensor_reduce(out=s, in_=e, axis=mybir.AxisListType.X,
                                op=add)
        rs = work.tile([128, 1], F32, tag="pirs", name="pirs")
        nc.vector.reciprocal(rs, s)
        pi = singles.tile([128, H], F32, tag=f"pi{i}", name=f"pi{i}")
        nc.vector.tensor_scalar_mul(pi, e, rs)
        sb_pi.append(pi)

    # ---- main loop over b-blocks (head-outer within each half) ----
    exp_scale = (1.0 / EMB_SCALE) if FP8 else 1.0

    def load_half(q):
        """DMA one emb half into SBUF (used by all 4 heads of the block)."""
        qoff = H0W if q else 0
        qw = H1W if q else H0W
        if FP8:
            eh = estream.tile([128, KT, SLOTW], FP8E4, tag="eh", name="eh")
            for kd in range(KT):
                nc.sync.dma_start(
                    out=eh[:, kd, :qw],
                    in_=embT[:, kd * V_S + qoff: kd * V_S + qoff + qw])
        else:
            eh = [estream.tile([128, SLOTW], BF16, tag=f"eh{kd}",
                               name=f"eh{kd}") for kd in range(KT)]
            for kd in range(KT):
                nc.sync.dma_start(
                    out=eh[kd][:, :qw],
                    in_=embT[kd * 128:(kd + 1) * 128, qoff:qoff + qw])
        return eh

    def emit_head_run(i, h, q, ci_base, chunks, eh, eq):
        """matmuls+exp(+sums) for one head over one emb half (weights
        loaded once)."""
        for cidx, (c0, cw) in enumerate(chunks):
            ci = ci_base + cidx
            ps = next_ps()
            for ns in range((cw + 511) // 512):
                n0 = c0 + ns * 512
                nw = min(512, c0 + cw - n0)
                if FP8:
                    nc.tensor.matmul(
                        ps[:, ns * 512:ns * 512 + nw],
                        proj[h][:, :, i * 128:(i + 1) * 128],
                        eh[:, :, n0:n0 + nw],
                        start=True, stop=True,
                        perf_mode=mybir.MatmulPerfMode.DoubleRow,
                    )
                else:
                    for kc in range(KT):
                        nc.tensor.matmul(
                            ps[:, ns * 512:ns * 512 + nw],
                            proj[h][kc][:, i * 128:(i + 1) * 128],
                            eh[kc][:, n0:n0 + nw],
                            start=(kc == 0), stop=(kc == KT - 1),
                        )
            nc.scalar.activation(out=eq[h][:, c0:c0 + cw], in_=ps[:, :cw],
                                 func=Exp, scale=exp_scale,
                                 accum_out=sums_t[i % 2][:, h * NCH + ci:
                                                         h * NCH + ci + 1])

    sums_t = [work.tile([128, NCH * H], F32, tag=f"sums{j}",
                        name=f"sums{j}") for j in range(2)]

    pending = []  # deferred DVE mixture ops of the previous block

    def drain(n):
        for _ in range(min(n, len(pending))):
            pending.pop(0)()

    eh0_next = None
    for i in range(N_BBLK - 1):
        # --- half 0 (prefetched during the previous block's q1) ---
        eh0 = eh0_next if eh0_next is not None else load_half(0)
        eq0 = [ering.tile([128, SLOTW], BF16, tag="e", name=f"e{h}q0")
               for h in range(H)]
        eh1 = load_half(1)  # issue q1's DMA now: lands well before use
        for h in range(H):
            emit_head_run(i, h, 0, 0, CHUNKS0, eh0, eq0)
            drain((3, 2, 2, 2)[h])
        # --- half 1: reduces are emitted before the drains so the next
        # AllReduce trigger never queues behind AR-dependent mixture ops
        eq1 = [ering.tile([128, SLOTW], BF16, tag="e", name=f"e{h}q1")
               for h in range(H)]
        eh0_next = load_half(0)  # prefetch next block's q0
        s_loc = work.tile([128, H], F32, tag="sloc", name="sloc")
        cc_in = dram.tile([128, H], F32, tag="ccin", name="ccin")
        cc_out = dram.tile([128, H], F32, tag="ccout", name="ccout")
        for h in range(H):
            emit_head_run(i, h, 1, len(CHUNKS0), CHUNKS1, eh1, eq1)
            nc.vector.tensor_reduce(
                out=s_loc[:, h:h + 1],
                in_=sums_t[i % 2][:, h * NCH:(h + 1) * NCH],
                axis=mybir.AxisListType.X, op=add)
            if h == H - 1:
                nc.gpsimd.dma_start(out=cc_in[:], in_=s_loc)
                nc.gpsimd.collective_compute(
                    "AllReduce", add,
                    replica_groups=[list(range(N_CORES))],
                    ins=[cc_in.opt()], outs=[cc_out.opt()],
                )
            drain((2, 2, 2, 2)[h])
        drain(len(pending))  # safety: should be empty already
        s_glob = work.tile([128, H], F32, tag="sglob", name="sglob")
        nc.gpsimd.dma_start(out=s_glob, in_=cc_out[:])
        # w = pi / S_glob on DVE, deferred (first AR-dependent ops in the
        # DVE FIFO are emitted ~1 chunk into the next block)
        rS = work.tile([128, H], F32, tag="rS", name="rS")
        w = work.tile([128, H], F32, tag="w", name="w")

        def op_w(rS=rS, w=w, s_glob=s_glob, pi=sb_pi[i]):
            nc.vector.reciprocal(rS, s_glob)
            nc.vector.tensor_mul(w, pi, rS)

        # --- mixture for this block: deferred into next block's stream ---
        # All in-place on the ring slots: scale each e_h by w_h (4x mode),
        # then accumulate into the head-3 slot with tensor_tensor adds (2x).
        def make_mix(i, w, eq, qw, goff):
            acc = eq[H - 1]

            def mul(h):
                def op():
                    nc.vector.tensor_scalar_mul(eq[h][:, :qw], eq[h][:, :qw],
                                                w[:, h:h + 1])
                return op

            def madd(h):
                def op():
                    nc.vector.tensor_tensor(out=acc[:, :qw], in0=acc[:, :qw],
                                            in1=eq[h][:, :qw], op=add)
                return op

            def op_dma():
                nc.sync.dma_start(
                    out=out[i * 128:(i + 1) * 128, goff:goff + qw],
                    in_=acc[:, :qw])
            return [mul(0), mul(H - 1), madd(0), mul(1), madd(1),
                    mul(2), madd(2), op_dma]

        # out-DMAs (sync HWDGE) sit at drain positions where their
        # mixture-waits are already satisfied at queue-head: no
        # head-of-line blocking of the emb stream
        pending = ([op_w] + make_mix(i, w, eq0, H0W, 0)
                   + make_mix(i, w, eq1, H1W, H0W))

    # ---- final block: head-major with per-head AllReduce, so only the
    # last head's collective + one scale/add/DMA remain after the last exp
    i = N_BBLK - 1
    eh0 = eh0_next
    eh1 = load_half(1)
    eq0, eq1 = [None] * H, [None] * H
    mix7 = []  # deferred per-head mixture ops (keep reduces at FIFO head)

    def emit_mix7(h, w7):
        for eq, qw in ((eq0, H0W), (eq1, H1W)):
            def op_mul(eq=eq, qw=qw, h=h, w7=w7):
                nc.vector.tensor_scalar_mul(eq[h][:, :qw], eq[h][:, :qw],
                                            w7)
            mix7.append(op_mul)
        if h > 0:
            for eq, qw in ((eq0, H0W), (eq1, H1W)):
                def op_add(eq=eq, qw=qw, h=h):
                    nc.vector.tensor_tensor(out=eq[h][:, :qw],
                                            in0=eq[h][:, :qw],
                                            in1=eq[h - 1][:, :qw], op=add)
                mix7.append(op_add)

    for h in range(H):
        eq0[h] = ering.tile([128, SLOTW], BF16, tag="e", name=f"f{h}q0")
        eq1[h] = ering.tile([128, SLOTW], BF16, tag="e", name=f"f{h}q1")
        emit_head_run(i, h, 0, 0, CHUNKS0, eh0, eq0)
        emit_head_run(i, h, 1, len(CHUNKS0), CHUNKS1, eh1, eq1)
        # issue this head's AllReduce trigger chain immediately; the
        # mixture ops run behind later heads' reduces in the DVE FIFO
        sl = work.tile([128, 1], F32, tag=f"sl7_{h}", name=f"sl7_{h}",
                       bufs=1)
        nc.vector.tensor_reduce(
            out=sl, in_=sums_t[i % 2][:, h * NCH:(h + 1) * NCH],
            axis=mybir.AxisListType.X, op=add)
        cc7i = dram.tile([128, 1], F32, tag=f"cc7i{h}", name=f"cc7i{h}",
                         bufs=1)
        cc7o = dram.tile([128, 1], F32, tag=f"cc7o{h}", name=f"cc7o{h}",
                         bufs=1)
        nc.gpsimd.dma_start(out=cc7i[:], in_=sl)
        nc.gpsimd.collective_compute(
            "AllReduce", add,
            replica_groups=[list(range(N_CORES))],
            ins=[cc7i.opt()], outs=[cc7o.opt()],
        )
        sg = work.tile([128, 1], F32, tag=f"sg7{h}", name=f"sg7{h}", bufs=1)
        rs7 = work.tile([128, 1], F32, tag=f"rs7{h}", name=f"rs7{h}", bufs=1)
        w7 = work.tile([128, 1], F32, tag=f"w7{h}", name=f"w7{h}", bufs=1)

        def op_w7(rs7=rs7, w7=w7, sg=sg, cc7o=cc7o, h=h):
            nc.gpsimd.dma_start(out=sg, in_=cc7o[:])
            nc.vector.reciprocal(rs7, sg)
            nc.vector.tensor_mul(w7, sb_pi[N_BBLK - 1][:, h:h + 1], rs7)
        mix7.append(op_w7)
        emit_mix7(h, w7)
        drain(5)  # block 6's deferred mixture
        # flush older heads' mixture ops (their ARs are already done);
        # keep only this head's ops pending so the next head's reduce
        # stays near the DVE FIFO head
        if h < H - 1:
            keep = 3 if h == 0 else 5
            while len(mix7) > keep:
                mix7.pop(0)()
    for op in mix7:
        op()
    nc.sync.dma_start(out=out[i * 128:(i + 1) * 128, 0:H0W],
                      in_=eq0[H - 1][:, :H0W])
    nc.sync.dma_start(out=out[i * 128:(i + 1) * 128, H0W:V_S],
                      in_=eq1[H - 1][:, :H1W])
    drain(len(pending))  # safety


def _get_nc():
    if "nc" not in _CACHE:
        _CACHE["nc"] = _build()
    return _CACHE["nc"]


def kernel(x, proj_mat, mix_mat, emb):
    nc = _get_nc()
    bf = ml_dtypes.bfloat16
    xT = np.ascontiguousarray(x.astype(bf).T)
    pmT = np.ascontiguousarray(proj_mat.astype(bf).T)
    mmT = np.ascontiguousarray(mix_mat.astype(bf).T)
    in_maps = []
    for c in range(N_CORES):
        shard = emb[c * V_S:(c + 1) * V_S]
        if FP8:
            # [dl, kd*V_S + v] = emb[v, kd*128+dl] * EMB_SCALE, fp8e4
            e16 = (shard.T * EMB_SCALE).astype(ml_dtypes.float8_e4m3)
            embT = np.ascontiguousarray(
                e16.reshape(KT, 128, V_S).transpose(1, 0, 2).reshape(
                    128, KT * V_S))
        else:
            embT = np.ascontiguousarray(shard.astype(bf).T)
        in_maps.append({"xT": xT, "pmT": pmT, "mmT": mmT, "embT": embT})
    res = run_bass_kernel_spmd(nc, in_maps, list(range(N_CORES)),
                               **_RUN_KWARGS)
    _CACHE["last_result"] = res
    return np.concatenate(
        [res.results[c]["out"].astype(np.float32) for c in range(N_CORES)],
        axis=1)


# revision 45
# speedup vs baseline: 1.0248x; 1.0003x over previous
"""Mixture-of-Softmaxes kernel for 8 Trainium2 NeuronCores.

Strategy: tensor-parallel over the vocab dimension (V=100000 -> 12500/core).
Each core computes all B rows for its vocab shard. Per 128-row block:
per-head logits via fp8e4 DoubleRow matmuls (K=256 in one PE pass), exp on
ScalarE (the pacing engine, ~154G elem/s) with fused per-chunk row-sum
accumulators, ONE [128,4] AllReduce per block of the softmax denominators,
then an in-place DVE mixture (tensor_scalar_mul at 4x + tensor_tensor adds
at 2x) accumulating into the head-3 ring slot. Output gathered on host.

Key structure:
- emb is streamed from DRAM per half-block (head-outer loop: weights stay
  loaded across each head's chunks), freeing SBUF for a 13-slot ring of
  exp tiles so the per-block collective + mixture overlap the next block's
  compute; emb halves are prefetched one phase ahead.
- 8 AllReduces (one [128,4]/block) + 3 prologue warmup collectives (the
  first ~3 collectives pay one-time setup). The final block instead uses
  per-head [128,1] AllReduces so only the last head's collective + one
  scale/add/DMA remain after the last exp.
- DVE-FIFO discipline: row-sum reduces (which feed the next AllReduce
  trigger) are emitted ahead of the previous block's AR-gated mixture ops;
  big out-DMAs ride the sync HWDGE queue at positions where their waits
  are already satisfied, keeping both the emb stream and the gpsimd
  collective-staging queue free of head-of-line blocking.
- projT (tanh) is written by ScalarE directly as DoubleRow-interleaved
  fp8 [128, 2, B] tiles and stays SBUF-resident.

Host-side prep: inputs transposed (contraction dim -> partitions); emb
pre-scaled by 16 and cast to fp8e4 (descaled for free via the exp's scale
argument); x/proj_mat/mix_mat in bf16; output bf16 -> f32 cast + vocab
concat on host.
"""

import numpy as np
import ml_dtypes

import concourse.bass as bass
import concourse.mybir as mybir
import concourse.tile as tile
from concourse import bacc
from concourse.bass_utils import run_bass_kernel_spmd
from concourse.bass_interp import get_hw_module

B, H, D, V = 1024, 4, 256, 100000
N_CORES = 8
V_S = V // N_CORES          # 12500 vocab entries per core
KT = D // 128               # 2 contraction k-tiles
N_BBLK = B // 128           # 8 b-blocks
H0W, H1W = 6144, 6356       # uneven halves of V_S (3x2048 | 3x2048+212)
SLOTW = H1W                 # ring slot width
E_SLOTS = 13                # exp ring slots (halves)
# (offset, width): two [128,2048] psum buffers ping-pong (global parity)
CHUNKS0 = [(0, 2048), (2048, 2048), (4096, 2048)]
CHUNKS1 = [(0, 2048), (2048, 2048), (4096, 2048), (6144, 212)]
NCH = len(CHUNKS0) + len(CHUNKS1)  # chunks (= accum cols) per head

FP8 = True                  # fp8e4 DoubleRow matmul for the big GEMM
DVE_SUMS = False            # row-sums on DVE (else ScalarE accum_out)
EMB_SCALE = 16.0            # host pre-scale of emb (undone in exp's scale)

F32 = mybir.dt.float32
BF16 = mybir.dt.bfloat16
FP8E4 = mybir.dt.float8e4

_RUN_KWARGS = {}  # test harness may set trace/tmpdir here
_CACHE = {}


def _build():
    nc = bacc.Bacc("TRN2", target_bir_lowering=False, debug=False,
                   num_devices=N_CORES)
    xT = nc.dram_tensor("xT", [D, B], BF16, kind="ExternalInput").ap()
    pmT = nc.dram_tensor("pmT", [D, H * D], BF16, kind="ExternalInput").ap()
    mmT = nc.dram_tensor("mmT", [D, H], BF16, kind="ExternalInput").ap()
    if FP8:
        embT = nc.dram_tensor("embT", [128, KT * V_S], FP8E4,
                              kind="ExternalInput").ap()
    else:
        embT = nc.dram_tensor("embT", [D, V_S], BF16, kind="ExternalInput").ap()
    out = nc.dram_tensor("out", [B, V_S], BF16, kind="ExternalOutput").ap()

    with tile.TileContext(nc) as tc:
        _body(tc, xT, pmT, mmT, embT, out)
        tc._pool_ctx.close()

    nc.compile()
    nc.m = get_hw_module(nc.m)
    return nc


def _body(tc, xT, pmT, mmT, embT, out):
    nc = tc.nc
    Exp = mybir.ActivationFunctionType.Exp
    Tanh = mybir.ActivationFunctionType.Tanh
    add = mybir.AluOpType.add
    mult = mybir.AluOpType.mult

    import contextlib
    ctx = contextlib.ExitStack()
    tc._pool_ctx = ctx
    singles = ctx.enter_context(tc.tile_pool(name="singles", bufs=1))
    work = ctx.enter_context(tc.tile_pool(name="work", bufs=2))
    ering = ctx.enter_context(tc.tile_pool(name="ering", bufs=E_SLOTS))
    estream = ctx.enter_context(tc.tile_pool(name="estream", bufs=2))
    psum = ctx.enter_context(tc.tile_pool(name="psum", bufs=2, space="PSUM"))
    dram = ctx.enter_context(tc.tile_pool(name="dram", bufs=2, space="DRAM"))

    # ---- warm up the CC stream: the first ~3 collectives otherwise pay
    # ~20-40us of one-time setup on the critical path ----
    zz = work.tile([128, H], F32, tag="zz", name="zz")
    nc.gpsimd.memset(zz, 0.0)
    for wi in range(3):
        warm_in = dram.tile([128, H], F32, tag=f"wrmin{wi}",
                            name=f"wrmin{wi}", bufs=1)
        warm_out = dram.tile([128, H], F32, tag=f"wrmout{wi}",
                             name=f"wrmout{wi}", bufs=1)
        nc.gpsimd.dma_start(out=warm_in[:], in_=zz)
        nc.gpsimd.collective_compute(
            "AllReduce", add,
            replica_groups=[list(range(N_CORES))],
            ins=[warm_in.opt()], outs=[warm_out.opt()],
        )

    # ---- prologue: resident inputs ----
    sb_xT, sb_pmT, sb_mmT = [], [], []
    for k in range(KT):
        t = singles.tile([128, B], BF16, tag=f"xT{k}", name=f"xT{k}")
        nc.sync.dma_start(out=t, in_=xT[k * 128:(k + 1) * 128, :])
        sb_xT.append(t)
        t = singles.tile([128, H * D], BF16, tag=f"pmT{k}", name=f"pmT{k}")
        nc.sync.dma_start(out=t, in_=pmT[k * 128:(k + 1) * 128, :])
        sb_pmT.append(t)
        t = singles.tile([128, H], BF16, tag=f"mmT{k}", name=f"mmT{k}")
        nc.sync.dma_start(out=t, in_=mmT[k * 128:(k + 1) * 128, :])
        sb_mmT.append(t)

    ps_parity = [0]

    def next_ps():
        pstag = "psA" if ps_parity[0] % 2 == 0 else "psB"
        ps_parity[0] += 1
        return psum.tile([128, 2048], F32, tag=pstag, name=pstag, bufs=1)

    # ---- projT = tanh(proj_mat @ x.T), resident (fp8 interleaved or bf16)
    if FP8:
        proj = [singles.tile([128, KT, B], FP8E4, tag=f"pj{h}", name=f"pj{h}")
                for h in range(H)]
    else:
        proj = [[singles.tile([128, B], BF16, tag=f"pj{h}_{kd}",
                              name=f"pj{h}_{kd}") for kd in range(KT)]
                for h in range(H)]
    for h in range(H):
        for kd in range(KT):
            for bs in range(B // 512):
                ps = next_ps()
                for kc in range(KT):
                    nc.tensor.matmul(
                        ps[:, :512],
                        sb_pmT[kc][:, h * D + kd * 128: h * D + (kd + 1) * 128],
                        sb_xT[kc][:, bs * 512:(bs + 1) * 512],
                        start=(kc == 0), stop=(kc == KT - 1),
                    )
                dst = (proj[h][:, kd, bs * 512:(bs + 1) * 512] if FP8
                       else proj[h][kd][:, bs * 512:(bs + 1) * 512])
                nc.scalar.activation(out=dst, in_=ps[:, :512], func=Tanh)

    # ---- pi[b, h] = softmax_h(x @ mix_mat.T) per b-block ----
    sb_pi = []
    for i in range(N_BBLK):
        ps = next_ps()
        for kc in range(KT):
            nc.tensor.matmul(
                ps[:, :H],
                sb_xT[kc][:, i * 128:(i + 1) * 128],
                sb_mmT[kc],
                start=(kc == 0), stop=(kc == KT - 1),
            )
        m = work.tile([128, 1], F32, tag="pim", name="pim")
        nc.vector.tensor_reduce(out=m, in_=ps[:, :H],
                                axis=mybir.AxisListType.X,
                                op=mybir.AluOpType.max)
        negm = work.tile([128, 1], F32, tag="pinegm", name="pinegm")
        nc.vector.tensor_scalar_mul(negm, m, -1.0)
        e = work.tile([128, H], F32, tag="pie", name="pie")
        nc.scalar.activation(out=e, in_=ps[:, :H], func=Exp, bias=negm)
        s = work.tile([128, 1], F32, tag="pis", name="pis")
        nc.vector.tensor_reduce(out=s, in_=e, axis=mybir.AxisListType.X,
                                op=add)
        rs = work.tile([128, 1], F32, tag="pirs", name="pirs")
        nc.vector.reciprocal(rs, s)
        pi = singles.tile([128, H], F32, tag=f"pi{i}", name=f"pi{i}")
        nc.vector.tensor_scalar_mul(pi, e, rs)
        sb_pi.append(pi)

    # ---- main loop over b-blocks (head-outer within each half) ----
    exp_scale = (1.0 / EMB_SCALE) if FP8 else 1.0

    def load_half(q):
        """DMA one emb half into SBUF (used by all 4 heads of the block)."""
        qoff = H0W if q else 0
        qw = H1W if q else H0W
        if FP8:
            eh = estream.tile([128, KT, SLOTW], FP8E4, tag="eh", name="eh")
            for kd in range(KT):
                nc.sync.dma_start(
                    out=eh[:, kd, :qw],
                    in_=embT[:, kd * V_S + qoff: kd * V_S + qoff + qw])
        else:
            eh = [estream.tile([128, SLOTW], BF16, tag=f"eh{kd}",
                               name=f"eh{kd}") for kd in range(KT)]
            for kd in range(KT):
                nc.sync.dma_start(
                    out=eh[kd][:, :qw],
                    in_=embT[kd * 128:(kd + 1) * 128, qoff:qoff + qw])
        return eh

    def emit_head_run(i, h, q, ci_base, chunks, eh, eq):
        """matmuls+exp(+sums) for one head over one emb half (weights
        loaded once)."""
        for cidx, (c0, cw) in enumerate(chunks):
            ci = ci_base + cidx
            ps = next_ps()
            for ns in range((cw + 511) // 512):
                n0 = c0 + ns * 512
                nw = min(512, c0 + cw - n0)
                if FP8:
                    nc.tensor.matmul(
                        ps[:, ns * 512:ns * 512 + nw],
                        proj[h][:, :, i * 128:(i + 1) * 128],
                        eh[:, :, n0:n0 + nw],
                        start=True, stop=True,
                        perf_mode=mybir.MatmulPerfMode.DoubleRow,
                    )
                else:
                    for kc in range(KT):
                        nc.tensor.matmul(
                            ps[:, ns * 512:ns * 512 + nw],
                            proj[h][kc][:, i * 128:(i + 1) * 128],
                            eh[kc][:, n0:n0 + nw],
                            start=(kc == 0), stop=(kc == KT - 1),
                        )
            nc.scalar.activation(out=eq[h][:, c0:c0 + cw], in_=ps[:, :cw],
                                 func=Exp, scale=exp_scale,
                                 accum_out=sums_t[i % 2][:, h * NCH + ci:
                                                         h * NCH + ci + 1])

    sums_t = [work.tile([128, NCH * H], F32, tag=f"sums{j}",
                        name=f"sums{j}") for j in range(2)]

    pending = []  # deferred DVE mixture ops of the previous block

    def drain(n):
        for _ in range(min(n, len(pending))):
            pending.pop(0)()

    eh0_next = None
    for i in range(N_BBLK - 1):
        # --- half 0 (prefetched during the previous block's q1) ---
        eh0 = eh0_next if eh0_next is not None else load_half(0)
        eq0 = [ering.tile([128, SLOTW], BF16, tag="e", name=f"e{h}q0")
               for h in range(H)]
        eh1 = load_half(1)  # issue q1's DMA now: lands well before use
        for h in range(H):
            emit_head_run(i, h, 0, 0, CHUNKS0, eh0, eq0)
            drain((3, 2, 2, 2)[h])
        # --- half 1: reduces are emitted before the drains so the next
        # AllReduce trigger never queues behind AR-dependent mixture ops
        eq1 = [ering.tile([128, SLOTW], BF16, tag="e", name=f"e{h}q1")
               for h in range(H)]
        eh0_next = load_half(0)  # prefetch next block's q0
        s_loc = work.tile([128, H], F32, tag="sloc", name="sloc")
        cc_in = dram.tile([128, H], F32, tag="ccin", name="ccin")
        cc_out = dram.tile([128, H], F32, tag="ccout", name="ccout")
        for h in range(H):
            emit_head_run(i, h, 1, len(CHUNKS0), CHUNKS1, eh1, eq1)
            nc.vector.tensor_reduce(
                out=s_loc[:, h:h + 1],
                in_=sums_t[i % 2][:, h * NCH:(h + 1) * NCH],
                axis=mybir.AxisListType.X, op=add)
            if h == H - 1:
                nc.gpsimd.dma_start(out=cc_in[:], in_=s_loc)
                nc.gpsimd.collective_compute(
                    "AllReduce", add,
                    replica_groups=[list(range(N_CORES))],
                    ins=[cc_in.opt()], outs=[cc_out.opt()],
                )
            drain((2, 2, 2, 2)[h])
        drain(len(pending))  # safety: should be empty already
        s_glob = work.tile([128, H], F32, tag="sglob", name="sglob")
        nc.gpsimd.dma_start(out=s_glob, in_=cc_out[:])
        # w = pi / S_glob on DVE, deferred (first AR-dependent ops in the
        # DVE FIFO are emitted ~1 chunk into the next block)
        rS = work.tile([128, H], F32, tag="rS", name="rS")
        w = work.tile([128, H], F32, tag="w", name="w")

        def op_w(rS=rS, w=w, s_glob=s_glob, pi=sb_pi[i]):
            nc.vector.reciprocal(rS, s_glob)
            nc.vector.tensor_mul(w, pi, rS)

        # --- mixture for this block: deferred into next block's stream ---
        # All in-place on the ring slots: scale each e_h by w_h (4x mode),
        # then accumulate into the head-3 slot with tensor_tensor adds (2x).
        def make_mix(i, w, eq, qw, goff):
            acc = eq[H - 1]

            def mul(h):
                def op():
                    nc.vector.tensor_scalar_mul(eq[h][:, :qw], eq[h][:, :qw],
                                                w[:, h:h + 1])
                return op

            def madd(h):
                def op():
                    nc.vector.tensor_tensor(out=acc[:, :qw], in0=acc[:, :qw],
                                            in1=eq[h][:, :qw], op=add)
                return op

            def op_dma():
                nc.sync.dma_start(
                    out=out[i * 128:(i + 1) * 128, goff:goff + qw],
                    in_=acc[:, :qw])
            return [mul(0), mul(H - 1), madd(0), mul(1), madd(1),
                    mul(2), madd(2), op_dma]

        # out-DMAs (sync HWDGE) sit at drain positions where their
        # mixture-waits are already satisfied at queue-head: no
        # head-of-line blocking of the emb stream
        pending = ([op_w] + make_mix(i, w, eq0, H0W, 0)
                   + make_mix(i, w, eq1, H1W, H0W))

    # ---- final block: head-major with per-head AllReduce, so only the
    # last head's collective + one scale/add/DMA remain after the last exp
    i = N_BBLK - 1
    eh0 = eh0_next
    eh1 = load_half(1)
    eq0, eq1 = [None] * H, [None] * H
    mix7 = []  # deferred per-head mixture ops (keep reduces at FIFO head)

    def emit_mix7(h, w7):
        for eq, qw in ((eq0, H0W), (eq1, H1W)):
            def op_mul(eq=eq, qw=qw, h=h, w7=w7):
                nc.vector.tensor_scalar_mul(eq[h][:, :qw], eq[h][:, :qw],
                                            w7)
            mix7.append(op_mul)
        if h > 0:
            for eq, qw in ((eq0, H0W), (eq1, H1W)):
                def op_add(eq=eq, qw=qw, h=h):
                    nc.vector.tensor_tensor(out=eq[h][:, :qw],
                                            in0=eq[h][:, :qw],
                                            in1=eq[h - 1][:, :qw], op=add)
                mix7.append(op_add)

    for h in range(H):
        eq0[h] = ering.tile([128, SLOTW], BF16, tag="e", name=f"f{h}q0")
        eq1[h] = ering.tile([128, SLOTW], BF16, tag="e", name=f"f{h}q1")
        emit_head_run(i, h, 0, 0, CHUNKS0, eh0, eq0)
        emit_head_run(i, h, 1, len(CHUNKS0), CHUNKS1, eh1, eq1)
        # issue this head's AllReduce trigger chain immediately; the
        # mixture ops run behind later heads' reduces in the DVE FIFO
        sl = work.tile([128, 1], F32, tag=f"sl7_{h}", name=f"sl7_{h}",
                       bufs=1)
        nc.vector.tensor_reduce(
            out=sl, in_=sums_t[i % 2][:, h * NCH:(h + 1) * NCH],
            axis=mybir.AxisListType.X, op=add)
        cc7i = dram.tile([128, 1], F32, tag=f"cc7i{h}", name=f"cc7i{h}",
                         bufs=1)
        cc7o = dram.tile([128, 1], F32, tag=f"cc7o{h}", name=f"cc7o{h}",
                         bufs=1)
        nc.gpsimd.dma_start(out=cc7i[:], in_=sl)
        nc.gpsimd.collective_compute(
            "AllReduce", add,
            replica_groups=[list(range(N_CORES))],
            ins=[cc7i.opt()], outs=[cc7o.opt()],
        )
        sg = work.tile([128, 1], F32, tag=f"sg7{h}", name=f"sg7{h}", bufs=1)
        rs7 = work.tile([128, 1], F32, tag=f"rs7{h}", name=f"rs7{h}", bufs=1)
        w7 = work.tile([128, 1], F32, tag=f"w7{h}", name=f"w7{h}", bufs=1)

        def op_w7(rs7=rs7, w7=w7, sg=sg, cc7o=cc7o, h=h):
            nc.gpsimd.dma_start(out=sg, in_=cc7o[:])
            nc.vector.reciprocal(rs7, sg)
            nc.vector.tensor_mul(w7, sb_pi[N_BBLK - 1][:, h:h + 1], rs7)
        mix7.append(op_w7)
        emit_mix7(h, w7)
        drain(5)  # block 6's deferred mixture
        # flush older heads' mixture ops (their ARs are already done);
        # keep only this head's ops pending so the next head's reduce
        # stays near the DVE FIFO head
        if h < H - 1:
            keep = 3 if h == 0 else 5
            while len(mix7) > keep:
                mix7.pop(0)()
    for op in mix7:
        op()
    nc.sync.dma_start(out=out[i * 128:(i + 1) * 128, 0:H0W],
                      in_=eq0[H - 1][:, :H0W])
    nc.sync.dma_start(out=out[i * 128:(i + 1) * 128, H0W:V_S],
                      in_=eq1[H - 1][:, :H1W])
    drain(len(pending))  # safety


def _get_nc():
    if "nc" not in _CACHE:
        _CACHE["nc"] = _build()
    return _CACHE["nc"]


def kernel(x, proj_mat, mix_mat, emb):
    nc = _get_nc()
    bf = ml_dtypes.bfloat16
    xT = np.ascontiguousarray(x.astype(bf).T)
    pmT = np.ascontiguousarray(proj_mat.astype(bf).T)
    mmT = np.ascontiguousarray(mix_mat.astype(bf).T)
    in_maps = []
    for c in range(N_CORES):
        shard = emb[c * V_S:(c + 1) * V_S]
        if FP8:
            # [dl, kd*V_S + v] = emb[v, kd*128+dl] * EMB_SCALE, fp8e4
            e16 = (shard.T * EMB_SCALE).astype(ml_dtypes.float8_e4m3)
            embT = np.ascontiguousarray(
                e16.reshape(KT, 128, V_S).transpose(1, 0, 2).reshape(
                    128, KT * V_S))
        else:
            embT = np.ascontiguousarray(shard.astype(bf).T)
        in_maps.append({"xT": xT, "pmT": pmT, "mmT": mmT, "embT": embT})
    res = run_bass_kernel_spmd(nc, in_maps, list(range(N_CORES)),
                               **_RUN_KWARGS)
    _CACHE["last_result"] = res
    return np.concatenate(
        [res.results[c]["out"].astype(np.float32) for c in range(N_CORES)],
        axis=1)
